# revision 15
# baseline (speedup 1.0000x reference)
"""Chunked-causal attention with sinks on 8 TRN2 NeuronCores.

Sharding: the 64 (batch, head) pairs are split 8-per-core (data parallel on
B, tensor parallel on H). Each core runs the same Bass program over its 8
pairs x 4 chunks of 1024 tokens.

The per-core shard layout is chosen for DMA/TensorE efficiency:
  - Q, K arrive pre-transposed as bf16 [pairs, D, S]: the score matmul
    contracts over D, which must sit on SBUF partitions, and bf16 is the
    matmul compute dtype either way (the host conversion is numerically
    identical to an on-device cast). Per-partition rows are contiguous.
  - V arrives as bf16 [pairs, P, nch, T, D+1] (s = t*P + p within a chunk),
    with a ones column appended: partition-major so each partition's slice
    is one contiguous DRAM run, and the ones column makes the PV matmul
    emit the softmax denominator as output column D.
  - The output is stored partition-major bf16 [pairs, nch, P, T, D] and
    un-permuted (and upcast to fp32) on the host.

Per (pair, chunk) the kernel computes, entirely on-chip:
  S_T[k, q] = K @ Q^T          (TensorE, bf16; scores transposed so that the
                                PV matmul can consume exp(S_T) directly)
  P_T       = exp(S_T / sqrt(D))  (split across two engines: ScalarE exact
                                exp for the key tiles that dominate few-key
                                rows, VectorE fast exp2-bitcast approximation
                                for the rest; softmax is shift-invariant and
                                scores here are O(5), so no max-subtraction)
  O[q, :]   = P_T^T @ [V | 1]  (TensorE; the ones column yields the softmax
                                denominator in column D of the same matmul)
  out       = O[:, :D] / (O[:, D] + exp(sink))

The VectorE exp uses the classic exponent-bits trick: for y = x*log2(e),
the bf16 bit pattern (127 + y) * 2^7 (computed as one fused mult+add
tensor_scalar with int16 output, then reinterpreted as bf16) equals
2^floor(y) * (1 + frac(y)) ~= 2^y, within +-4.3% before the balancing
constant. Those relative errors wash out in the softmax ratio for rows
with many keys; all key tiles whose diagonal block serves rows with <=256
keys stay on ScalarE's exact exp.

The emission is software-pipelined: chunk c+1's score groups are woven
between chunk c's PV pairs so the PE always has issueable matmuls while
the exp engines drain, and each engine's strict-FIFO queue receives its
ops in the order their inputs become ready (ScalarE carries only exps and
the cheap denominator adds; VectorE carries the approx exps plus the
reciprocal/normalize tail, emitted after the next chunk's exps). The
per-half-chunk PSUM accumulators (2 banks each, double buffered) are
normalized and released mid-chunk so the next chunk's PV never waits on
a full-chunk epilogue. DMA loads are issued one full chunk ahead.

(Tried and rejected: fp8-e4m3 DoubleRow score matmuls - the 2x PE win is
real and the Ki=64 row-tiled form works on hardware, but the ~4% rms
score quantization noise turns into up to ~3.5% output error against the
2e-2 budget because attention rows concentrate their weight mass on few
keys; fp8 V fails the same way on large-|v| elements.)
"""

import ml_dtypes
import numpy as np

import concourse.bacc as bacc
import concourse.bass as bass
import concourse.mybir as mybir
import concourse.tile as tile
from concourse.bass_utils import run_bass_kernel_spmd

N_CORES = 8
B, S, H, D = 4, 4096, 16, 128
C = 1024                # chunk size
NCH = S // C            # chunks per sequence
PAIRS = B * H           # 64 (batch, head) pairs
PPC = PAIRS // N_CORES  # pairs per core
P = 128                 # SBUF partitions
T = C // P              # 128-row tiles per chunk
SCALE = 1.0 / float(np.sqrt(D))

F32 = mybir.dt.float32
BF16 = mybir.dt.bfloat16
I16 = mybir.dt.int16

# exp2-bitcast (Schraudolph) constants for the VectorE exp: the bf16 bits of
# exp(s*SCALE) are approximately s*EXPA + EXPB when computed as an integer.
EXPA = float(SCALE * np.log2(np.e) * 128.0)
# 16256 = 127 << 7 (bf16 exponent bias); -7.6 balances the piecewise-linear
# overshoot of (1+f) vs 2^f so the relative error is centered.
EXPB = 16256.0 - 7.6


def _build_program(ppc=PPC, nch=NCH):
    s_len = nch * C
    nc = bacc.Bacc("TRN2", target_bir_lowering=False, debug=False)
    qt_d = nc.dram_tensor("qt", [ppc, D, s_len], BF16, kind="ExternalInput")
    kt_d = nc.dram_tensor("kt", [ppc, D, s_len], BF16, kind="ExternalInput")
    v_d = nc.dram_tensor("v", [ppc, P, nch, T, D + 1], BF16, kind="ExternalInput")
    es_d = nc.dram_tensor("esink", [P, ppc], F32, kind="ExternalInput")
    out_d = nc.dram_tensor("out", [ppc, P, nch, T, D], BF16, kind="ExternalOutput")

    with tile.TileContext(nc) as tc:
        with (
            tc.tile_pool(name="loads", bufs=4) as loads,
            tc.tile_pool(name="ptile", bufs=4) as ppool,
            tc.tile_pool(name="outs", bufs=3) as opool,
            tc.tile_pool(name="small", bufs=4) as small,
            tc.tile_pool(name="spsum", bufs=2, space="PSUM") as spsum,
            tc.tile_pool(name="opsum", bufs=2, space="PSUM") as opsum,
        ):
            # Key-tile groups packed so each group's scores/exp span is one
            # contiguous <=1024-column region (5 exp calls instead of 8).
            GROUPS = [[0], [1, 7], [2, 6], [3, 5], [4]]
            # Per-group exp engine split: (vector-engine columns, from col 0).
            # VectorE uses the exp2-bitcast trick; its key tiles' diagonal
            # blocks only serve rows with >=257 keys, where the ~4%
            # per-weight error averages out. Split to balance engine load;
            # ScalarE stays a pure exp queue so no late-dependency op ever
            # blocks a ready exp in its FIFO.
            VE_COLS = {0: 0, 1: 0, 2: 1024, 3: 0, 4: 512}
            WIDTH = {kt: C - P * kt for kt in range(T)}
            OFF = {}
            GSPAN = []
            for gi, g in enumerate(GROUPS):
                goff = C * gi
                w = 0
                for kt in g:
                    OFF[kt] = goff + w
                    w += WIDTH[kt]
                GSPAN.append((goff, w))
            PTW = C * (len(GROUPS) - 1) + GSPAN[-1][1]

            def emit_scores_group(gi, qsrc, ksrc, pt_flat, qsplit=None):
                goff, gw = GSPAN[gi]
                st = spsum.tile([P, C], F32, tag="st")
                for kt in GROUPS[gi]:
                    c0 = kt * P
                    poff = OFF[kt] - goff  # packed col of q = c0
                    # split matmuls at PSUM bank boundaries (packed col 512)
                    # and, for the cold chunk, at the q source-tile boundary
                    spans = []
                    a = c0
                    while a < C:
                        pa = poff + (a - c0)
                        room = 512 - pa % 512
                        b_ = min(a + min(room, 512), C)
                        if qsplit is not None and a < qsplit < b_:
                            b_ = qsplit
                        spans.append((a, b_, pa))
                        a = b_
                    for a, b_, pa in spans:
                        nc.tensor.matmul(
                            st[:, pa:pa + (b_ - a)],
                            ksrc(kt),
                            qsrc(a, b_),
                            start=True,
                            stop=True,
                        )
                vw = VE_COLS[gi]
                if vw:
                    # exp(s*SCALE) via exponent-bits construction on VectorE
                    nc.vector.tensor_scalar(
                        pt_flat[:, goff:goff + vw].bitcast(I16),
                        st[:, 0:vw],
                        EXPA,
                        EXPB,
                        op0=mybir.AluOpType.mult,
                        op1=mybir.AluOpType.add,
                    )
                if vw < gw:
                    nc.scalar.activation(
                        pt_flat[:, goff + vw:goff + gw],
                        st[:, vw:gw],
                        mybir.ActivationFunctionType.Exp,
                        scale=SCALE,
                    )
                for kt in GROUPS[gi]:
                    # zero the strictly-upper (k > q) part of the diag block
                    nc.gpsimd.affine_select(
                        out=pt_flat[:, OFF[kt]:OFF[kt] + P],
                        in_=pt_flat[:, OFF[kt]:OFF[kt] + P],
                        compare_op=mybir.AluOpType.is_ge,
                        fill=0.0,
                        base=0,
                        channel_multiplier=-1,
                        pattern=[[1, P]],
                    )

            def emit_pv_pair(j, vb, pt_flat, oacc):
                # PV accumulation for query tiles 2j, 2j+1 into a half-chunk
                # 2-bank PSUM accumulator (jj = j % 2 selects the bank).
                # Each [P, 129] matmul output stays inside one 2KB bank.
                jj = j % 2
                for qq in range(2):
                    qt = 2 * j + qq
                    for kt in range(qt + 1):
                        nc.tensor.matmul(
                            oacc[:, jj, 129 * qq:129 * qq + 129],
                            pt_flat[:, OFF[kt] + (qt - kt) * P:
                                    OFF[kt] + (qt - kt + 1) * P],
                            vb[:, kt, :],
                            start=(kt == 0),
                            stop=(kt == qt),
                        )

            def emit_finish_half(h, oacc, es_t, osb):
                # denominator + reciprocal + normalize (all VectorE) for
                # query tiles 4h..4h+3; frees the 2-bank accumulator early so
                # the next chunk's PV can start without waiting on the tail.
                den = small.tile([P, 4], F32, tag="den")
                den_in = bass.AP(
                    tensor=oacc.tensor,
                    offset=oacc.offset + 128,
                    ap=[oacc.ap[0], [512, 2], [129, 2]],
                )
                den4 = bass.AP(
                    tensor=den.tensor,
                    offset=den.offset,
                    ap=[den.ap[0], [2, 2], [1, 2]],
                )
                # den split across engines: ScalarE carries three of the five
                # exp groups so it is the fuller queue; the h=0 den moves to
                # VectorE (tensor_tensor add with the es column broadcast
                # along the free axis) to shave its ~265ns off ScalarE.
                if h == 0:
                    es_b4 = bass.AP(
                        tensor=es_t.tensor,
                        offset=es_t.offset,
                        ap=[es_t.ap[0], [0, 2], [0, 2]],
                    )
                    nc.vector.tensor_tensor(
                        den4, den_in, es_b4, mybir.AluOpType.add
                    )
                else:
                    nc.scalar.activation(
                        den4,
                        den_in,
                        mybir.ActivationFunctionType.Identity,
                        bias=es_t,
                    )
                rec = small.tile([P, 4], F32, tag="rec")
                nc.vector.reciprocal(rec, den)
                oacc_in = bass.AP(
                    tensor=oacc.tensor,
                    offset=oacc.offset,
                    ap=[oacc.ap[0], [512, 2], [129, 2], [1, 128]],
                )
                rec_b = bass.AP(
                    tensor=rec.tensor,
                    offset=rec.offset,
                    ap=[rec.ap[0], [2, 2], [1, 2], [0, 128]],
                )
                osb_out = bass.AP(
                    tensor=osb.tensor,
                    offset=osb.offset + h * 512,
                    ap=[osb.ap[0], [256, 2], [128, 2], [1, 128]],
                )
                nc.vector.tensor_tensor(
                    osb_out, oacc_in, rec_b, mybir.AluOpType.mult
                )

            def emit_finish_quarter(h, jj, oacc, es_t, osb):
                # last-chunk tail: finish one PV pair (2 query tiles) as soon
                # as its bank is done, so the post-matmul serial chain is a
                # 256-col normalize instead of a 512-col half.
                den = small.tile([P, 2], F32, tag="den2")
                den_in = bass.AP(
                    tensor=oacc.tensor,
                    offset=oacc.offset + jj * 512 + 128,
                    ap=[oacc.ap[0], [129, 2]],
                )
                nc.scalar.activation(
                    den, den_in, mybir.ActivationFunctionType.Identity,
                    bias=es_t,
                )
                rec = small.tile([P, 2], F32, tag="rec2")
                nc.vector.reciprocal(rec, den)
                oacc_in = bass.AP(
                    tensor=oacc.tensor,
                    offset=oacc.offset + jj * 512,
                    ap=[oacc.ap[0], [129, 2], [1, 128]],
                )
                rec_b = bass.AP(
                    tensor=rec.tensor,
                    offset=rec.offset,
                    ap=[rec.ap[0], [1, 2], [0, 128]],
                )
                osb_out = bass.AP(
                    tensor=osb.tensor,
                    offset=osb.offset + (h * 4 + jj * 2) * 128,
                    ap=[osb.ap[0], [128, 2], [1, 128]],
                )
                nc.vector.tensor_tensor(
                    osb_out, oacc_in, rec_b, mybir.AluOpType.mult
                )

            # ---- software-pipelined schedule over the 32 chunks ----
            # Chunk c's five score groups (+ exps + masks) are interleaved
            # between chunk c-1's PV pairs so the PE always has issueable
            # matmuls while the exp engines drain, and each engine's FIFO
            # receives ops in the order their inputs become ready.
            n_chunks = ppc * nch
            state = {"dma_prio": 0}  # per-chunk tiles

            def dma_pri(out, in_):
                # Load DMAs get strictly increasing priorities 0,1,2,... in
                # emission order, far below every compute priority. The Tile
                # scheduler pops ready instructions per engine from a
                # priority heap, so this makes the Sync queue issue loads as
                # early as buffer recycling allows AND in exactly this order
                # (a flat high_priority() block would tie them all at 0 and
                # scramble the order, starving the cold chunk).
                with tc.high_priority(offset=tc.cur_priority - state["dma_prio"]):
                    nc.sync.dma_start(out=out, in_=in_)
                state["dma_prio"] += 1

            def ensure_loads(c):
                # Cold start: chunks 0 and 1 get dedicated single-DMA tiles.
                # Splitting a shared tile across several DMAs serializes them
                # on the tile's semaphore (each must wait for the previous
                # one's completion so waiters can attribute increments), which
                # lets later-issued prefetch DMAs jump ahead in the in-order
                # DMA queue and starve chunk 0 (an ~11us PE gap). Separate
                # tiles -> separate semaphores -> all cold loads issue
                # back-to-back, smallest/most-critical first.
                if c == 0:
                    if ("cold0",) in state:
                        return
                    k00 = loads.tile([P, P], BF16, tag="k00", bufs=1)
                    dma_pri(k00, kt_d[0, :, 0:P])
                    q0a = loads.tile([P, 512], BF16, tag="q0a", bufs=1)
                    dma_pri(q0a, qt_d[0, :, 0:512])
                    q0b = loads.tile([P, 512], BF16, tag="q0b", bufs=1)
                    dma_pri(q0b, qt_d[0, :, 512:C])
                    k0r = loads.tile([P, C - P], BF16, tag="k0r", bufs=1)
                    dma_pri(k0r, kt_d[0, :, P:C])
                    v0 = loads.tile([P, T, D + 1], BF16, tag="v0", bufs=1)
                    dma_pri(v0, v_d[0, :, 0])
                    es_t = small.tile([P, ppc], F32, tag="esink", bufs=1)
                    dma_pri(es_t, es_d[:, :])
                    state[("es",)] = es_t
                    state[("cold0",)] = (k00, k0r, q0a, q0b, v0)
                    return
                if c == 1:
                    if ("cold1",) in state:
                        return
                    k1 = loads.tile([P, C], BF16, tag="k1", bufs=1)
                    dma_pri(k1, kt_d[0, :, C:2 * C])
                    q1 = loads.tile([P, C], BF16, tag="q1", bufs=1)
                    dma_pri(q1, qt_d[0, :, C:2 * C])
                    v1 = loads.tile([P, T, D + 1], BF16, tag="v1", bufs=1)
                    dma_pri(v1, v_d[0, :, 1])
                    state[("cold1",)] = (k1, q1, v1)
                    return
                pair, ch = divmod(c, nch)
                half, chsub = divmod(ch, 2)
                if chsub != 0 or ("ld", pair, half) in state:
                    return
                h0 = half * 2 * C
                qtb2 = loads.tile([P, 2 * C], BF16, tag="qtb")
                ktb2 = loads.tile([P, 2 * C], BF16, tag="ktb")
                vb2 = loads.tile([P, 2, T, D + 1], BF16, tag="vb")
                dma_pri(qtb2, qt_d[pair, :, h0:h0 + 2 * C])
                dma_pri(ktb2, kt_d[pair, :, h0:h0 + 2 * C])
                dma_pri(vb2, v_d[pair, :, 2 * half:2 * half + 2])
                state[("ld", pair, half)] = (qtb2, ktb2, vb2)

            def chunk_inputs(c):
                pair, ch = divmod(c, nch)
                half, chsub = divmod(ch, 2)
                ensure_loads(c)
                if c == 0:
                    k00, k0r, q0a, q0b, v0 = state[("cold0",)]
                    qsrc = lambda a, b: (q0a[:, a:b] if b <= 512
                                         else q0b[:, a - 512:b - 512])
                    ksrc = lambda kt: (k00[:, 0:P] if kt == 0
                                       else k0r[:, (kt - 1) * P:kt * P])
                    vb = v0
                elif c == 1:
                    k1, q1, v1 = state[("cold1",)]
                    qsrc = lambda a, b: q1[:, a:b]
                    ksrc = lambda kt: k1[:, kt * P:(kt + 1) * P]
                    vb = v1
                else:
                    qtb2, ktb2, vb2 = state[("ld", pair, half)]
                    o = chsub * C
                    qsrc = lambda a, b: qtb2[:, o + a:o + b]
                    ksrc = lambda kt: ktb2[:, o + kt * P:o + (kt + 1) * P]
                    vb = vb2[:, chsub]
                return (qsrc, ksrc, vb, state[("es",)][:, pair:pair + 1],
                        pair, ch)

            def emit_sc(c, gi):
                qsrc, ksrc, vb, es_t, pair, ch = state[("in", c)]
                emit_scores_group(gi, qsrc, ksrc, state[("pt", c)],
                                  qsplit=512 if c == 0 else None)

            def emit_pv(c, j):
                qsrc, ksrc, vb, es_t, pair, ch = state[("in", c)]
                oacc = state[("oa", c)][j // 2]
                emit_pv_pair(j, vb, state[("pt", c)], oacc)

            def emit_fin(c, h):
                qsrc, ksrc, vb, es_t, pair, ch = state[("in", c)]
                emit_finish_half(h, state[("oa", c)][h], es_t, state[("osb", c)])

            def open_chunk(c):
                state[("in", c)] = chunk_inputs(c)
                state[("pt", c)] = ppool.tile([P, PTW], BF16, tag="pt", name="pt")
                state[("osb", c)] = opool.tile([P, T, D], BF16, tag="osb", name="osb")
                state[("oa", c)] = (
                    opsum.tile([P, 2, 512], F32, tag="oacc", name="oacc0"),
                    opsum.tile([P, 2, 512], F32, tag="oacc", name="oacc1"),
                )

            def close_chunk(c):
                qsrc, ksrc, vb, es_t, pair, ch = state.pop(("in", c))
                nc.sync.dma_start(
                    out=out_d[pair, :, ch], in_=state.pop(("osb", c))
                )
                state.pop(("pt", c))
                state.pop(("oa", c))

            open_chunk(0)
            for gi in (0, 2, 1, 3, 4):
                emit_sc(0, gi)
            for c in range(n_chunks):
                nxt = c + 1 if c + 1 < n_chunks else None
                if nxt is not None:
                    open_chunk(nxt)
                if c + 2 < n_chunks:
                    # issue DMA loads one full round ahead of first use
                    ensure_loads(c + 2)
                emit_pv(c, 0)
                if nxt is not None:
                    emit_sc(nxt, 0)
                    emit_sc(nxt, 2)
                emit_pv(c, 1)
                emit_pv(c, 2)
                emit_fin(c, 0)
                if nxt is not None:
                    emit_sc(nxt, 1)
                    emit_sc(nxt, 3)
                emit_pv(c, 3)
                if nxt is not None:
                    emit_sc(nxt, 4)
                    emit_fin(c, 1)
                    close_chunk(c)
                else:
                    # last chunk: finish at PV-pair (quarter) granularity and
                    # store in pieces so the post-matmul serial chain is as
                    # short as possible
                    qsrc, ksrc, vb, es_t, pair, ch = state.pop(("in", c))
                    osb = state.pop(("osb", c))
                    oacc1 = state[("oa", c)][1]
                    emit_finish_quarter(1, 0, oacc1, es_t, osb)
                    nc.sync.dma_start(
                        out=out_d[pair, :, ch, 0:4], in_=osb[:, 0:4]
                    )
                    nc.sync.dma_start(
                        out=out_d[pair, :, ch, 4:6], in_=osb[:, 4:6]
                    )
                    emit_finish_quarter(1, 1, oacc1, es_t, osb)
                    nc.sync.dma_start(
                        out=out_d[pair, :, ch, 6:8], in_=osb[:, 6:8]
                    )
                    state.pop(("pt", c))
                    state.pop(("oa", c))

    nc.compile()
    return nc


_PROGRAM = None


def _get_program():
    global _PROGRAM
    if _PROGRAM is None:
        _PROGRAM = _build_program()
    return _PROGRAM


def _prep_in_maps(q, k, v, sinks):
    # [B,S,H,D] -> [B*H, S, D]
    qp = np.ascontiguousarray(q.transpose(0, 2, 1, 3)).reshape(PAIRS, S, D)
    kp = np.ascontiguousarray(k.transpose(0, 2, 1, 3)).reshape(PAIRS, S, D)
    vp = np.ascontiguousarray(v.transpose(0, 2, 1, 3)).reshape(PAIRS, S, D)
    # Q, K additionally transposed to [pairs, D, S] bf16 (matmul layout/dtype)
    qT = np.ascontiguousarray(qp.transpose(0, 2, 1)).astype(ml_dtypes.bfloat16)
    kT = np.ascontiguousarray(kp.transpose(0, 2, 1)).astype(ml_dtypes.bfloat16)
    # V: bf16, partition-major [pairs, P, nch, T, D+1] with a ones column
    vaug = np.empty((PAIRS, NCH, T, P, D + 1), dtype=ml_dtypes.bfloat16)
    vaug[..., :D] = vp.reshape(PAIRS, NCH, T, P, D).astype(ml_dtypes.bfloat16)
    vaug[..., D] = np.asarray(1.0, ml_dtypes.bfloat16)
    vaug = np.ascontiguousarray(vaug.transpose(0, 3, 1, 2, 4))
    es_pairs = np.tile(np.exp(sinks), B)  # es_pairs[i] = exp(sinks[i % H])

    in_maps = []
    for c in range(N_CORES):
        sl = slice(c * PPC, (c + 1) * PPC)
        # esink as [P, ppc]: one small DMA loads every pair's exp(sink)
        esb = np.ascontiguousarray(
            np.repeat(es_pairs[sl][None, :], P, axis=0).astype(np.float32)
        )
        in_maps.append(
            {"qt": qT[sl], "kt": kT[sl], "v": vaug[sl], "esink": esb}
        )
    return in_maps


def kernel(q, k, v, sinks, chunk_size):
    assert int(chunk_size) == C
    q = np.asarray(q, dtype=np.float32)
    k = np.asarray(k, dtype=np.float32)
    v = np.asarray(v, dtype=np.float32)
    sinks = np.asarray(sinks, dtype=np.float32)
    assert q.shape == (B, S, H, D)

    in_maps = _prep_in_maps(q, k, v, sinks)
    nc = _get_program()
    res = run_bass_kernel_spmd(nc, in_maps, core_ids=list(range(N_CORES)))

    outp = np.concatenate(
        [np.asarray(res.results[c]["out"]).astype(np.float32)
         for c in range(N_CORES)], axis=0
    )
    # [pairs, p, chunk, t, d] -> [pairs, s, d] (s = chunk*C + t*P + p)
    outp = outp.transpose(0, 2, 3, 1, 4).reshape(PAIRS, S, D)
    out = outp.reshape(B, H, S, D).transpose(0, 2, 1, 3)
    return np.ascontiguousarray(out)



# revision 16
# speedup vs baseline: 1.0183x; 1.0183x over previous
"""Chunked-causal attention with sinks on 8 TRN2 NeuronCores.

Sharding: the 64 (batch, head) pairs are split 8-per-core (data parallel on
B, tensor parallel on H). Each core runs the same Bass program over its 8
pairs x 4 chunks of 1024 tokens.

The per-core shard layout is chosen for DMA/TensorE efficiency:
  - Q, K arrive pre-transposed as bf16 [pairs, D, S]: the score matmul
    contracts over D, which must sit on SBUF partitions, and bf16 is the
    matmul compute dtype either way (the host conversion is numerically
    identical to an on-device cast). Per-partition rows are contiguous.
  - V arrives as bf16 [pairs, P, nch, T, D+1] (s = t*P + p within a chunk),
    with a ones column appended: partition-major so each partition's slice
    is one contiguous DRAM run, and the ones column makes the PV matmul
    emit the softmax denominator as output column D.
  - The output is stored partition-major bf16 [pairs, nch, P, T, D] and
    un-permuted (and upcast to fp32) on the host.

Per (pair, chunk) the kernel computes, entirely on-chip:
  S_T[k, q] = K @ Q^T          (TensorE, bf16; scores transposed so that the
                                PV matmul can consume exp(S_T) directly)
  P_T       = exp(S_T / sqrt(D))  (split across two engines: ScalarE exact
                                exp for the key tiles that dominate few-key
                                rows, VectorE fast exp2-bitcast approximation
                                for the rest; softmax is shift-invariant and
                                scores here are O(5), so no max-subtraction)
  O[q, :]   = P_T^T @ [V | 1]  (TensorE; the ones column yields the softmax
                                denominator in column D of the same matmul)
  out       = O[:, :D] / (O[:, D] + exp(sink))

The VectorE exp uses the classic exponent-bits trick: for y = x*log2(e),
the bf16 bit pattern (127 + y) * 2^7 (computed as one fused mult+add
tensor_scalar with int16 output, then reinterpreted as bf16) equals
2^floor(y) * (1 + frac(y)) ~= 2^y, within +-4.3% before the balancing
constant. Those relative errors wash out in the softmax ratio for rows
with many keys; all key tiles whose diagonal block serves rows with <=256
keys stay on ScalarE's exact exp.

The emission is software-pipelined: chunk c+1's score groups are woven
between chunk c's PV pairs so the PE always has issueable matmuls while
the exp engines drain, and each engine's strict-FIFO queue receives its
ops in the order their inputs become ready (ScalarE carries only exps and
the cheap denominator adds; VectorE carries the approx exps plus the
reciprocal/normalize tail, emitted after the next chunk's exps). The
per-half-chunk PSUM accumulators (2 banks each, double buffered) are
normalized and released mid-chunk so the next chunk's PV never waits on
a full-chunk epilogue. DMA loads are issued one full chunk ahead.

(Tried and rejected: fp8-e4m3 DoubleRow score matmuls - the 2x PE win is
real and the Ki=64 row-tiled form works on hardware, but the ~4% rms
score quantization noise turns into up to ~3.5% output error against the
2e-2 budget because attention rows concentrate their weight mass on few
keys; fp8 V fails the same way on large-|v| elements.)
"""

import ml_dtypes
import numpy as np

import concourse.bacc as bacc
import concourse.bass as bass
import concourse.mybir as mybir
import concourse.tile as tile
from concourse.bass_utils import run_bass_kernel_spmd

N_CORES = 8
B, S, H, D = 4, 4096, 16, 128
C = 1024                # chunk size
NCH = S // C            # chunks per sequence
PAIRS = B * H           # 64 (batch, head) pairs
PPC = PAIRS // N_CORES  # pairs per core
P = 128                 # SBUF partitions
T = C // P              # 128-row tiles per chunk
SCALE = 1.0 / float(np.sqrt(D))

F32 = mybir.dt.float32
BF16 = mybir.dt.bfloat16
I16 = mybir.dt.int16

# exp2-bitcast (Schraudolph) constants for the VectorE exp: the bf16 bits of
# exp(s*SCALE) are approximately s*EXPA + EXPB when computed as an integer.
EXPA = float(SCALE * np.log2(np.e) * 128.0)
# 16256 = 127 << 7 (bf16 exponent bias); -7.6 balances the piecewise-linear
# overshoot of (1+f) vs 2^f so the relative error is centered.
EXPB = 16256.0 - 7.6


def _build_program(ppc=PPC, nch=NCH):
    s_len = nch * C
    nc = bacc.Bacc("TRN2", target_bir_lowering=False, debug=False)
    qt_d = nc.dram_tensor("qt", [ppc, D, s_len], BF16, kind="ExternalInput")
    kt_d = nc.dram_tensor("kt", [ppc, D, s_len], BF16, kind="ExternalInput")
    v_d = nc.dram_tensor("v", [ppc, P, nch, T, D + 1], BF16, kind="ExternalInput")
    es_d = nc.dram_tensor("esink", [P, ppc], F32, kind="ExternalInput")
    out_d = nc.dram_tensor("out", [ppc, P, nch, T, D], BF16, kind="ExternalOutput")

    with tile.TileContext(nc) as tc:
        with (
            tc.tile_pool(name="loads", bufs=4) as loads,
            tc.tile_pool(name="ptile", bufs=4) as ppool,
            tc.tile_pool(name="outs", bufs=3) as opool,
            tc.tile_pool(name="small", bufs=4) as small,
            tc.tile_pool(name="spsum", bufs=2, space="PSUM") as spsum,
            tc.tile_pool(name="opsum", bufs=2, space="PSUM") as opsum,
        ):
            # Key-tile groups packed so each group's scores/exp span is one
            # contiguous <=1024-column region (5 exp calls instead of 8).
            GROUPS = [[0], [1, 7], [2, 6], [3, 5], [4]]
            # Per-group exp engine split: (vector-engine columns, from col 0).
            # VectorE uses the exp2-bitcast trick; its key tiles' diagonal
            # blocks only serve rows with >=257 keys, where the ~4%
            # per-weight error averages out. Split to balance engine load;
            # ScalarE stays a pure exp queue so no late-dependency op ever
            # blocks a ready exp in its FIFO.
            VE_COLS = {0: 0, 1: 0, 2: 1024, 3: 0, 4: 512}
            WIDTH = {kt: C - P * kt for kt in range(T)}
            OFF = {}
            GSPAN = []
            for gi, g in enumerate(GROUPS):
                goff = C * gi
                w = 0
                for kt in g:
                    OFF[kt] = goff + w
                    w += WIDTH[kt]
                GSPAN.append((goff, w))
            PTW = C * (len(GROUPS) - 1) + GSPAN[-1][1]

            def emit_scores_group(gi, qsrc, ksrc, pt_flat, qsplit=None):
                goff, gw = GSPAN[gi]
                st = spsum.tile([P, C], F32, tag="st")
                for kt in GROUPS[gi]:
                    c0 = kt * P
                    poff = OFF[kt] - goff  # packed col of q = c0
                    # split matmuls at PSUM bank boundaries (packed col 512)
                    # and, for the cold chunk, at the q source-tile boundary
                    spans = []
                    a = c0
                    while a < C:
                        pa = poff + (a - c0)
                        room = 512 - pa % 512
                        b_ = min(a + min(room, 512), C)
                        if qsplit is not None and a < qsplit < b_:
                            b_ = qsplit
                        spans.append((a, b_, pa))
                        a = b_
                    for a, b_, pa in spans:
                        nc.tensor.matmul(
                            st[:, pa:pa + (b_ - a)],
                            ksrc(kt),
                            qsrc(a, b_),
                            start=True,
                            stop=True,
                        )
                vw = VE_COLS[gi]
                if vw:
                    # exp(s*SCALE) via exponent-bits construction on VectorE
                    nc.vector.tensor_scalar(
                        pt_flat[:, goff:goff + vw].bitcast(I16),
                        st[:, 0:vw],
                        EXPA,
                        EXPB,
                        op0=mybir.AluOpType.mult,
                        op1=mybir.AluOpType.add,
                    )
                if vw < gw:
                    nc.scalar.activation(
                        pt_flat[:, goff + vw:goff + gw],
                        st[:, vw:gw],
                        mybir.ActivationFunctionType.Exp,
                        scale=SCALE,
                    )
                for kt in GROUPS[gi]:
                    # zero the strictly-upper (k > q) part of the diag block
                    nc.gpsimd.affine_select(
                        out=pt_flat[:, OFF[kt]:OFF[kt] + P],
                        in_=pt_flat[:, OFF[kt]:OFF[kt] + P],
                        compare_op=mybir.AluOpType.is_ge,
                        fill=0.0,
                        base=0,
                        channel_multiplier=-1,
                        pattern=[[1, P]],
                    )

            def emit_pv_pair(j, vb, pt_flat, oacc):
                # PV accumulation for query tiles 2j, 2j+1 into a half-chunk
                # 2-bank PSUM accumulator (jj = j % 2 selects the bank).
                # Each [P, 129] matmul output stays inside one 2KB bank.
                jj = j % 2
                for qq in range(2):
                    qt = 2 * j + qq
                    for kt in range(qt + 1):
                        nc.tensor.matmul(
                            oacc[:, jj, 129 * qq:129 * qq + 129],
                            pt_flat[:, OFF[kt] + (qt - kt) * P:
                                    OFF[kt] + (qt - kt + 1) * P],
                            vb[:, kt, :],
                            start=(kt == 0),
                            stop=(kt == qt),
                        )

            def emit_finish_half(h, oacc, es_t, osb):
                # denominator + reciprocal + normalize (all VectorE) for
                # query tiles 4h..4h+3; frees the 2-bank accumulator early so
                # the next chunk's PV can start without waiting on the tail.
                den = small.tile([P, 4], F32, tag="den")
                den_in = bass.AP(
                    tensor=oacc.tensor,
                    offset=oacc.offset + 128,
                    ap=[oacc.ap[0], [512, 2], [129, 2]],
                )
                den4 = bass.AP(
                    tensor=den.tensor,
                    offset=den.offset,
                    ap=[den.ap[0], [2, 2], [1, 2]],
                )
                # den on ScalarE: it has idle slack and this keeps the
                # VectorE queue free for the exp approximations
                nc.scalar.activation(
                    den4,
                    den_in,
                    mybir.ActivationFunctionType.Identity,
                    bias=es_t,
                )
                rec = small.tile([P, 4], F32, tag="rec")
                nc.vector.reciprocal(rec, den)
                oacc_in = bass.AP(
                    tensor=oacc.tensor,
                    offset=oacc.offset,
                    ap=[oacc.ap[0], [512, 2], [129, 2], [1, 128]],
                )
                rec_b = bass.AP(
                    tensor=rec.tensor,
                    offset=rec.offset,
                    ap=[rec.ap[0], [2, 2], [1, 2], [0, 128]],
                )
                osb_out = bass.AP(
                    tensor=osb.tensor,
                    offset=osb.offset + h * 512,
                    ap=[osb.ap[0], [256, 2], [128, 2], [1, 128]],
                )
                nc.vector.tensor_tensor(
                    osb_out, oacc_in, rec_b, mybir.AluOpType.mult
                )

            def emit_finish_quarter(h, jj, oacc, es_t, osb):
                # last-chunk tail: finish one PV pair (2 query tiles) as soon
                # as its bank is done, so the post-matmul serial chain is a
                # 256-col normalize instead of a 512-col half.
                den = small.tile([P, 2], F32, tag="den2")
                den_in = bass.AP(
                    tensor=oacc.tensor,
                    offset=oacc.offset + jj * 512 + 128,
                    ap=[oacc.ap[0], [129, 2]],
                )
                nc.scalar.activation(
                    den, den_in, mybir.ActivationFunctionType.Identity,
                    bias=es_t,
                )
                rec = small.tile([P, 2], F32, tag="rec2")
                nc.vector.reciprocal(rec, den)
                oacc_in = bass.AP(
                    tensor=oacc.tensor,
                    offset=oacc.offset + jj * 512,
                    ap=[oacc.ap[0], [129, 2], [1, 128]],
                )
                rec_b = bass.AP(
                    tensor=rec.tensor,
                    offset=rec.offset,
                    ap=[rec.ap[0], [1, 2], [0, 128]],
                )
                osb_out = bass.AP(
                    tensor=osb.tensor,
                    offset=osb.offset + (h * 4 + jj * 2) * 128,
                    ap=[osb.ap[0], [128, 2], [1, 128]],
                )
                nc.vector.tensor_tensor(
                    osb_out, oacc_in, rec_b, mybir.AluOpType.mult
                )

            # ---- software-pipelined schedule over the 32 chunks ----
            # Chunk c's five score groups (+ exps + masks) are interleaved
            # between chunk c-1's PV pairs so the PE always has issueable
            # matmuls while the exp engines drain, and each engine's FIFO
            # receives ops in the order their inputs become ready.
            n_chunks = ppc * nch
            state = {"dma_prio": 0}  # per-chunk tiles

            def dma_pri(out, in_):
                # Load DMAs get strictly increasing priorities 0,1,2,... in
                # emission order, far below every compute priority. The Tile
                # scheduler pops ready instructions per engine from a
                # priority heap, so this makes the Sync queue issue loads as
                # early as buffer recycling allows AND in exactly this order
                # (a flat high_priority() block would tie them all at 0 and
                # scramble the order, starving the cold chunk).
                with tc.high_priority(offset=tc.cur_priority - state["dma_prio"]):
                    nc.sync.dma_start(out=out, in_=in_)
                state["dma_prio"] += 1

            def ensure_loads(c):
                # Cold start: chunks 0 and 1 get dedicated single-DMA tiles.
                # Splitting a shared tile across several DMAs serializes them
                # on the tile's semaphore (each must wait for the previous
                # one's completion so waiters can attribute increments), which
                # lets later-issued prefetch DMAs jump ahead in the in-order
                # DMA queue and starve chunk 0 (an ~11us PE gap). Separate
                # tiles -> separate semaphores -> all cold loads issue
                # back-to-back, smallest/most-critical first.
                if c == 0:
                    if ("cold0",) in state:
                        return
                    k00 = loads.tile([P, P], BF16, tag="k00", bufs=1)
                    dma_pri(k00, kt_d[0, :, 0:P])
                    q0a = loads.tile([P, 512], BF16, tag="q0a", bufs=1)
                    dma_pri(q0a, qt_d[0, :, 0:512])
                    q0b = loads.tile([P, 512], BF16, tag="q0b", bufs=1)
                    dma_pri(q0b, qt_d[0, :, 512:C])
                    k0r = loads.tile([P, C - P], BF16, tag="k0r", bufs=1)
                    dma_pri(k0r, kt_d[0, :, P:C])
                    v0 = loads.tile([P, T, D + 1], BF16, tag="v0", bufs=1)
                    dma_pri(v0, v_d[0, :, 0])
                    es_t = small.tile([P, ppc], F32, tag="esink", bufs=1)
                    dma_pri(es_t, es_d[:, :])
                    state[("es",)] = es_t
                    state[("cold0",)] = (k00, k0r, q0a, q0b, v0)
                    return
                if c == 1:
                    if ("cold1",) in state:
                        return
                    k1 = loads.tile([P, C], BF16, tag="k1", bufs=1)
                    dma_pri(k1, kt_d[0, :, C:2 * C])
                    q1 = loads.tile([P, C], BF16, tag="q1", bufs=1)
                    dma_pri(q1, qt_d[0, :, C:2 * C])
                    v1 = loads.tile([P, T, D + 1], BF16, tag="v1", bufs=1)
                    dma_pri(v1, v_d[0, :, 1])
                    state[("cold1",)] = (k1, q1, v1)
                    return
                pair, ch = divmod(c, nch)
                half, chsub = divmod(ch, 2)
                if chsub != 0 or ("ld", pair, half) in state:
                    return
                h0 = half * 2 * C
                qtb2 = loads.tile([P, 2 * C], BF16, tag="qtb")
                ktb2 = loads.tile([P, 2 * C], BF16, tag="ktb")
                vb2 = loads.tile([P, 2, T, D + 1], BF16, tag="vb")
                dma_pri(qtb2, qt_d[pair, :, h0:h0 + 2 * C])
                dma_pri(ktb2, kt_d[pair, :, h0:h0 + 2 * C])
                dma_pri(vb2, v_d[pair, :, 2 * half:2 * half + 2])
                state[("ld", pair, half)] = (qtb2, ktb2, vb2)

            def chunk_inputs(c):
                pair, ch = divmod(c, nch)
                half, chsub = divmod(ch, 2)
                ensure_loads(c)
                if c == 0:
                    k00, k0r, q0a, q0b, v0 = state[("cold0",)]
                    qsrc = lambda a, b: (q0a[:, a:b] if b <= 512
                                         else q0b[:, a - 512:b - 512])
                    ksrc = lambda kt: (k00[:, 0:P] if kt == 0
                                       else k0r[:, (kt - 1) * P:kt * P])
                    vb = v0
                elif c == 1:
                    k1, q1, v1 = state[("cold1",)]
                    qsrc = lambda a, b: q1[:, a:b]
                    ksrc = lambda kt: k1[:, kt * P:(kt + 1) * P]
                    vb = v1
                else:
                    qtb2, ktb2, vb2 = state[("ld", pair, half)]
                    o = chsub * C
                    qsrc = lambda a, b: qtb2[:, o + a:o + b]
                    ksrc = lambda kt: ktb2[:, o + kt * P:o + (kt + 1) * P]
                    vb = vb2[:, chsub]
                return (qsrc, ksrc, vb, state[("es",)][:, pair:pair + 1],
                        pair, ch)

            def emit_sc(c, gi):
                qsrc, ksrc, vb, es_t, pair, ch = state[("in", c)]
                emit_scores_group(gi, qsrc, ksrc, state[("pt", c)],
                                  qsplit=512 if c == 0 else None)

            def emit_pv(c, j):
                qsrc, ksrc, vb, es_t, pair, ch = state[("in", c)]
                oacc = state[("oa", c)][j // 2]
                emit_pv_pair(j, vb, state[("pt", c)], oacc)

            def emit_fin(c, h):
                qsrc, ksrc, vb, es_t, pair, ch = state[("in", c)]
                emit_finish_half(h, state[("oa", c)][h], es_t, state[("osb", c)])

            def open_chunk(c):
                state[("in", c)] = chunk_inputs(c)
                state[("pt", c)] = ppool.tile([P, PTW], BF16, tag="pt", name="pt")
                state[("osb", c)] = opool.tile([P, T, D], BF16, tag="osb", name="osb")
                state[("oa", c)] = (
                    opsum.tile([P, 2, 512], F32, tag="oacc", name="oacc0"),
                    opsum.tile([P, 2, 512], F32, tag="oacc", name="oacc1"),
                )

            def close_chunk(c):
                qsrc, ksrc, vb, es_t, pair, ch = state.pop(("in", c))
                nc.sync.dma_start(
                    out=out_d[pair, :, ch], in_=state.pop(("osb", c))
                )
                state.pop(("pt", c))
                state.pop(("oa", c))

            open_chunk(0)
            for gi in (0, 2, 1, 3, 4):
                emit_sc(0, gi)
            for c in range(n_chunks):
                nxt = c + 1 if c + 1 < n_chunks else None
                if nxt is not None:
                    open_chunk(nxt)
                if c + 2 < n_chunks:
                    # issue DMA loads one full round ahead of first use
                    ensure_loads(c + 2)
                emit_pv(c, 0)
                if nxt is not None:
                    emit_sc(nxt, 0)
                    emit_sc(nxt, 2)
                emit_pv(c, 1)
                emit_pv(c, 2)
                emit_fin(c, 0)
                if nxt is not None:
                    emit_sc(nxt, 1)
                    emit_sc(nxt, 3)
                emit_pv(c, 3)
                if nxt is not None:
                    emit_sc(nxt, 4)
                    emit_fin(c, 1)
                    close_chunk(c)
                else:
                    # last chunk: finish at PV-pair (quarter) granularity and
                    # store in pieces so the post-matmul serial chain is as
                    # short as possible
                    qsrc, ksrc, vb, es_t, pair, ch = state.pop(("in", c))
                    osb = state.pop(("osb", c))
                    oacc1 = state[("oa", c)][1]
                    emit_finish_quarter(1, 0, oacc1, es_t, osb)
                    nc.sync.dma_start(
                        out=out_d[pair, :, ch, 0:4], in_=osb[:, 0:4]
                    )
                    nc.sync.dma_start(
                        out=out_d[pair, :, ch, 4:6], in_=osb[:, 4:6]
                    )
                    emit_finish_quarter(1, 1, oacc1, es_t, osb)
                    nc.sync.dma_start(
                        out=out_d[pair, :, ch, 6:8], in_=osb[:, 6:8]
                    )
                    state.pop(("pt", c))
                    state.pop(("oa", c))

    nc.compile()
    return nc


_PROGRAM = None


def _get_program():
    global _PROGRAM
    if _PROGRAM is None:
        _PROGRAM = _build_program()
    return _PROGRAM


def _prep_in_maps(q, k, v, sinks):
    # [B,S,H,D] -> [B*H, S, D]
    qp = np.ascontiguousarray(q.transpose(0, 2, 1, 3)).reshape(PAIRS, S, D)
    kp = np.ascontiguousarray(k.transpose(0, 2, 1, 3)).reshape(PAIRS, S, D)
    vp = np.ascontiguousarray(v.transpose(0, 2, 1, 3)).reshape(PAIRS, S, D)
    # Q, K additionally transposed to [pairs, D, S] bf16 (matmul layout/dtype)
    qT = np.ascontiguousarray(qp.transpose(0, 2, 1)).astype(ml_dtypes.bfloat16)
    kT = np.ascontiguousarray(kp.transpose(0, 2, 1)).astype(ml_dtypes.bfloat16)
    # V: bf16, partition-major [pairs, P, nch, T, D+1] with a ones column
    vaug = np.empty((PAIRS, NCH, T, P, D + 1), dtype=ml_dtypes.bfloat16)
    vaug[..., :D] = vp.reshape(PAIRS, NCH, T, P, D).astype(ml_dtypes.bfloat16)
    vaug[..., D] = np.asarray(1.0, ml_dtypes.bfloat16)
    vaug = np.ascontiguousarray(vaug.transpose(0, 3, 1, 2, 4))
    es_pairs = np.tile(np.exp(sinks), B)  # es_pairs[i] = exp(sinks[i % H])

    in_maps = []
    for c in range(N_CORES):
        sl = slice(c * PPC, (c + 1) * PPC)
        # esink as [P, ppc]: one small DMA loads every pair's exp(sink)
        esb = np.ascontiguousarray(
            np.repeat(es_pairs[sl][None, :], P, axis=0).astype(np.float32)
        )
        in_maps.append(
            {"qt": qT[sl], "kt": kT[sl], "v": vaug[sl], "esink": esb}
        )
    return in_maps


def kernel(q, k, v, sinks, chunk_size):
    assert int(chunk_size) == C
    q = np.asarray(q, dtype=np.float32)
    k = np.asarray(k, dtype=np.float32)
    v = np.asarray(v, dtype=np.float32)
    sinks = np.asarray(sinks, dtype=np.float32)
    assert q.shape == (B, S, H, D)

    in_maps = _prep_in_maps(q, k, v, sinks)
    nc = _get_program()
    res = run_bass_kernel_spmd(nc, in_maps, core_ids=list(range(N_CORES)))

    outp = np.concatenate(
        [np.asarray(res.results[c]["out"]).astype(np.float32)
         for c in range(N_CORES)], axis=0
    )
    # [pairs, p, chunk, t, d] -> [pairs, s, d] (s = chunk*C + t*P + p)
    outp = outp.transpose(0, 2, 3, 1, 4).reshape(PAIRS, S, D)
    out = outp.reshape(B, H, S, D).transpose(0, 2, 1, 3)
    return np.ascontiguousarray(out)

